# revision 55
# baseline (speedup 1.0000x reference)
"""Trainium2 Bass kernel for a dense transformer block.

reference: x -> LN1 -> 16-head causal attention (+residual) -> LN2 -> MLP
(+residual), x: [2, 2048, 1024] fp32.

Sharding: sequence-parallel with zigzag load balancing, zero collectives.
Core c (of 8) handles batch c//4 and query chunks j=c%4 and 7-j (256 rows
each => 512 rows/core). Each core recomputes LN1+K/V over a unified
2304-token kv space: the batch's first 1792 rows (longest strict prefix any
query block needs) plus the core's own 512 rows. Per-core validity is pure
data (aug score rows + additive causal masks), so one SPMD program runs on
all 8 cores.

All matmul operands are bf16 (PSUM accumulation stays fp32). Q/K/V
projections process head PAIRS with 128-wide stationary weights (matmul
cost depends only on the moving dim); per-head [64, n] halves are peeled
off PSUM by partition-shifted Act/DVE copies. V is projected token-major
in one pass for all heads; each head's stationary AV slice [tokens, 65]
carries an appended ones-column so softmax denominators ride the AV
matmul. Each PSUM bank holds exactly one accumulation group (subrange
matmuls with independent start flags drop the other subrange's partial
sums on HW). The attention inner loop is software-pipelined two chunks
deep; V-half-0 and pair-0's K projection are interleaved into the LN1
loop so the PE has work while the LN chain runs on DVE/Act.
"""

import sys

sys.path.insert(0, "/opt/trn_rl_repo")

from contextlib import ExitStack

import numpy as np

import concourse.bacc as bacc
import concourse.mybir as mybir
import concourse.tile as tile
from concourse.bass_utils import run_bass_kernel_spmd
from concourse.masks import make_identity

F32 = mybir.dt.float32
BF16 = mybir.dt.bfloat16
AF = mybir.ActivationFunctionType
ALU = mybir.AluOpType

B, P, D, H, DH = 2, 2048, 1024, 16, 64
FF = 4 * D
EPS = 1e-5
NCORES = 8
KV = 1792            # rect-path kv rows (longest strict prefix = 7*256)
QL = 512             # query rows per core
T = KV + QL          # unified kv token space: rect prefix + own rows
TC = T // 128        # 18 kv chunks
DC = D // 128        # 8 contraction chunks over D
FC = FF // 128       # 32 f-chunks
NPAIR = H // 2       # 8 head pairs
BIG = 30000.0        # additive mask magnitude; exp(-30000) == 0

# N-tiling of [*, T] projection outputs (PSUM bank is 512 fp32 wide)
NT = [(0, 512), (512, 512), (1024, 512), (1536, 512), (2048, 256)]


def build_nc():
    nc = bacc.Bacc(trn_type="TRN2")

    xin = nc.declare_dram_parameter("xin", [T, D], BF16, isOutput=False)
    identm = nc.declare_dram_parameter("identm", [128, 128], BF16, isOutput=False)
    xqbp = nc.declare_dram_parameter("xqbp", [QL, D], F32, isOutput=False)
    wq = nc.declare_dram_parameter("wq", [NPAIR, 128, DC * 128], BF16,
                                   isOutput=False)
    wk = nc.declare_dram_parameter("wk", [NPAIR, 128, DC * 128], BF16,
                                   isOutput=False)
    wv = nc.declare_dram_parameter("wv", [D, D], BF16, isOutput=False)
    bq = nc.declare_dram_parameter("bq", [128, NPAIR], F32, isOutput=False)
    wp = nc.declare_dram_parameter("wp", [D, D], BF16, isOutput=False)
    w1 = nc.declare_dram_parameter("w1", [FC, 128, DC * 128], BF16,
                                   isOutput=False)
    w2 = nc.declare_dram_parameter("w2", [FF, D], BF16, isOutput=False)
    b1v = nc.declare_dram_parameter("b1v", [FF], F32, isOutput=False)
    b2v = nc.declare_dram_parameter("b2v", [D], F32, isOutput=False)
    augq = nc.declare_dram_parameter("augq", [2, QL], BF16, isOutput=False)
    augk = nc.declare_dram_parameter("augk", [2, T], BF16, isOutput=False)
    dmask = nc.declare_dram_parameter("dmask", [4, 128, 256], F32, isOutput=False)
    out = nc.declare_dram_parameter("out", [QL, D], F32, isOutput=True)

    wv_v = wv.ap().rearrange("(dc p) e -> p dc e", p=128)
    wp_v = wp.ap().rearrange("(dc p) e -> p dc e", p=128)
    b1_v = b1v.ap().rearrange("(fc p) -> p fc", p=128)          # [128, 32]
    b2_v = b2v.ap().rearrange("(a d) -> a d", a=1)
    dm_v = dmask.ap().rearrange("c p n -> p c n")

    with tile.TileContext(nc) as tc, ExitStack() as ctx:
        persist = ctx.enter_context(tc.tile_pool(name="persist", bufs=1))
        spool = ctx.enter_context(tc.tile_pool(name="spool", bufs=3))

        # ---- constants (issued on the gpsimd DMA queue; SP queue stays
        # free for the x tiles that gate the LN pipeline)
        ident = persist.tile([128, 128], BF16)
        nc.gpsimd.dma_start(ident[:], identm.ap())
        eps_t = persist.tile([128, 1], F32)
        nc.vector.memset(eps_t[:], EPS)
        dm = persist.tile([128, 4, 256], F32)
        bq_sb = persist.tile([128, NPAIR], F32)
        b1_sb = persist.tile([128, FC], F32)
        b2_row = persist.tile([1, D], F32)
        b2_bc = persist.tile([128, D], F32)

        oT = persist.tile([128, NPAIR, QL], BF16)  # attention out, feature-major

        # attention-lifetime big tensors (pool closes before phase 3 so the
        # MLP phases get the SBUF back)
        attnbig = tc.tile_pool(name="attnbig", bufs=1)
        abp = attnbig.__enter__()

        # kp/qp: per-head tiles, k/q at partitions 0:64, aug rows at 64:66.
        # 2 slots each way -> pair p uses slot p%2; aug rows written once.
        kp_t = [[abp.tile([66, T], BF16, name=f"kp{s}{ab}") for ab in "AB"]
                for s in range(2)]
        qp_t = [[abp.tile([66, QL], BF16, name=f"qp{s}{ab}") for ab in "AB"]
                for s in range(2)]

        # token-major V for all heads: per pair a 130-wide block
        # [vA(64) | ones | vB(64) | ones]; head slices [0:65] / [65:130].
        vp = abp.tile([128, TC, NPAIR, 130], BF16)
        nc.vector.memset(vp[:, :, :, 64:65], 1.0)
        nc.vector.memset(vp[:, :, :, 129:130], 1.0)

        hT = abp.tile([128, DC, T], BF16)  # LN1(x) transposed

        # phase-1-resident weights (gpsimd queue, prefetched at t=0)
        wv0_t = abp.tile([128, DC, 512], BF16)
        nc.gpsimd.dma_start(wv0_t[:], wv_v[:, :, 0:512])
        wv1_t = abp.tile([128, DC, 512], BF16)
        wp_sb = persist.tile([128, DC, D], BF16)
        wk0_t = abp.tile([128, DC, 128], BF16)
        nc.gpsimd.dma_start(wk0_t[:].rearrange("p a b -> p (a b)"), wk.ap()[0])
        wq0_t = abp.tile([128, DC, 128], BF16)
        nc.gpsimd.dma_start(wq0_t[:].rearrange("p a b -> p (a b)"), wq.ap()[0])

        def ln_tile(src, dst):
            """dst = (src - mean) * rsqrt(var + EPS); stats on DVE, the
            normalize itself on Act (per-partition scale/bias)."""
            stats = spool.tile([128, 2, nc.vector.BN_STATS_DIM], F32,
                               tag="ln_stats")
            for sg in range(2):
                nc.vector.bn_stats(out=stats[:, sg, :],
                                   in_=src[:, sg * 512:(sg + 1) * 512])
            mv = spool.tile([128, nc.vector.BN_AGGR_DIM], F32, tag="ln_mv")
            nc.vector.bn_aggr(out=mv[:], in_=stats[:])
            rstd = spool.tile([128, 1], F32, tag="ln_rstd")
            nc.scalar.activation(out=rstd[:], in_=mv[:, 1:2],
                                 func=AF.Abs_reciprocal_sqrt, bias=eps_t[:])
            mb = spool.tile([128, 1], F32, tag="ln_mb")
            nc.vector.scalar_tensor_tensor(out=mb[:], in0=mv[:, 0:1],
                                           scalar=-1.0, in1=rstd[:],
                                           op0=ALU.mult, op1=ALU.mult)
            nc.scalar.activation(out=dst, in_=src, func=AF.Identity,
                                 bias=mb[:], scale=rstd[:])

        # ===== attention-phase pools (PSUM: 2+2+3+1 = 8 banks) =====
        with tc.tile_pool(name="xpool", bufs=3) as xpool, \
             tc.tile_pool(name="hpool", bufs=2) as hpool, \
             tc.tile_pool(name="wvp", bufs=2) as wvp, \
             tc.tile_pool(name="wqkp", bufs=4) as wqkp, \
             tc.tile_pool(name="apool", bufs=4) as apool, \
             tc.tile_pool(name="rpool", bufs=2) as rpool, \
             tc.tile_pool(name="trps", bufs=1, space="PSUM") as trps, \
             tc.tile_pool(name="projps", bufs=2, space="PSUM") as projps, \
             tc.tile_pool(name="spsum", bufs=3, space="PSUM") as spsum, \
             tc.tile_pool(name="opsum", bufs=2, space="PSUM") as opsum:

            def v_chunk(tk, wv_t, h, on_act=False):
                """token-major V projection for one 128-token chunk."""
                ps = projps.tile([128, 512], F32, tag="projps")
                for dc in range(DC):
                    nc.tensor.matmul(ps[:], hT[:, dc, 128 * tk:128 * (tk + 1)],
                                     wv_t[:, dc, :],
                                     start=(dc == 0), stop=(dc == DC - 1))
                srcA = ps[:, 0:256].rearrange("p (a c) -> p a c", a=4)
                srcB = ps[:, 256:512].rearrange("p (a c) -> p a c", a=4)
                dstA = vp[:, tk, 4 * h:4 * h + 4, 0:64]
                dstB = vp[:, tk, 4 * h:4 * h + 4, 65:129]
                if on_act:
                    nc.scalar.copy(dstA, srcA)
                    nc.scalar.copy(dstB, srcB)
                else:
                    nc.vector.tensor_copy(dstA, srcA)
                    nc.vector.tensor_copy(dstB, srcB)

            def k_nt(ti, wk_t, kpA, kpB):
                """K projection (both heads of a pair) for one NT tile."""
                n0, nl = NT[ti]
                ps = projps.tile([128, 512], F32, tag="projps")
                for dc in range(DC):
                    nc.tensor.matmul(ps[:, 0:nl], wk_t[:, dc, :],
                                     hT[:, dc, n0:n0 + nl],
                                     start=(dc == 0), stop=(dc == DC - 1))
                nc.vector.tensor_copy(kpA[0:64, n0:n0 + nl], ps[0:64, 0:nl])
                nc.vector.tensor_copy(kpB[0:64, n0:n0 + nl], ps[64:128, 0:nl])

            def q_proj(p, wq_t, qpA, qpB):
                ps = projps.tile([128, 512], F32, tag="projps")
                for dc in range(DC):
                    nc.tensor.matmul(ps[:], wq_t[:, dc, :], hT[:, dc, KV:T],
                                     start=(dc == 0), stop=(dc == DC - 1))
                nc.scalar.activation(qpA[0:64, :], ps[0:64, :], AF.Identity,
                                     bias=bq_sb[0:64, p:p + 1])
                nc.scalar.activation(qpB[0:64, :], ps[64:128, :], AF.Identity,
                                     bias=bq_sb[64:128, p:p + 1])

            def attention(p, kp, qp, hb):
                """one head: scores -> exp -> AV, pipelined 2 chunks deep."""
                vsl = vp[:, :, p, 0:65] if hb == 0 else vp[:, :, p, 65:130]
                ops = opsum.tile([65, 512], F32, tag="ops")

                # steps: 6 single 512-wide chunks, then 256-wide chunks
                # fused in pairs (one sps bank / one exp per pair)
                STEPS = [0, 1, 2, 3, 4, 5,
                         (6, 7), (8, 9), (10, 11), (12, 13),
                         (14, 15), (16, 17)]

                def score_exp(it):
                    sps = spsum.tile([128, 512], F32, tag="sps")
                    att = apool.tile([128, 512], BF16, tag="att")
                    if isinstance(it, int):
                        nc.tensor.matmul(sps[:],
                                         kp[:, 128 * it:128 * (it + 1)],
                                         qp[:, 0:512], start=True, stop=True)
                    else:
                        k1, k2 = it
                        qo = 0 if k1 == 14 else 256
                        nc.tensor.matmul(sps[:, 0:256],
                                         kp[:, 128 * k1:128 * (k1 + 1)],
                                         qp[:, qo:qo + 256],
                                         start=True, stop=True,
                                         skip_group_check=True)
                        nc.tensor.matmul(sps[:, 256:512],
                                         kp[:, 128 * k2:128 * (k2 + 1)],
                                         qp[:, qo:qo + 256],
                                         start=True, stop=True,
                                         skip_group_check=True)
                        if k1 >= 14:
                            nc.vector.tensor_add(
                                sps[:], sps[:],
                                dm[:, k1 - 14:k1 - 12, :].rearrange(
                                    "p a n -> p (a n)"))
                    nc.scalar.activation(att[:], sps[:], AF.Exp)
                    return att

                def av(it, att):
                    if isinstance(it, int):
                        nc.tensor.matmul(ops[:, 0:512], vsl[:, it, :],
                                         att[:, 0:512],
                                         start=(it == 0), stop=False,
                                         skip_group_check=True)
                    else:
                        k1, k2 = it
                        co = 0 if k1 == 14 else 256
                        nc.tensor.matmul(ops[:, co:co + 256], vsl[:, k1, :],
                                         att[:, 0:256],
                                         start=False, stop=False,
                                         skip_group_check=True)
                        nc.tensor.matmul(ops[:, co:co + 256], vsl[:, k2, :],
                                         att[:, 256:512],
                                         start=False, stop=(k2 == 17),
                                         skip_group_check=True)

                atts = {i: score_exp(STEPS[i]) for i in range(3)}
                for si in range(len(STEPS)):
                    if si + 3 < len(STEPS):
                        atts[si + 3] = score_exp(STEPS[si + 3])
                    av(STEPS[si], atts.pop(si))

                rec = rpool.tile([1, QL], F32, tag="rec")
                nc.vector.reciprocal(rec[0:1, 0:256], ops[64:65, 0:256])
                nc.vector.reciprocal(rec[0:1, 256:512], ops[64:65, 256:512])
                sbc = rpool.tile([64, QL], F32, tag="sbc")
                nc.gpsimd.partition_broadcast(sbc[:], rec[0:1, :])
                nc.vector.tensor_mul(oT[hb * 64:hb * 64 + 64, p, :],
                                     ops[0:64, :], sbc[:])

            # pair-weight tiles; DMAs are issued mid-phase-1 (gated below)
            wk_ts = {0: wk0_t}
            wq_ts = {0: wq0_t}
            for p in range(1, NPAIR):
                wk_ts[p] = wqkp.tile([128, DC, 128], BF16, tag="wk_t",
                                     name=f"wk_t{p}")
                wq_ts[p] = wqkp.tile([128, DC, 128], BF16, tag="wq_t",
                                     name=f"wq_t{p}")

            # ===== Phase 1: LN1 -> hT, interleaved with V-half-0 and
            # pair-0's K projection so the PE is fed during the LN chain
            KNT_AT = {3: 0, 7: 1, 11: 2, 15: 3}
            for i in range(TC):
                xt = xpool.tile([128, D], BF16, tag="xt")
                nc.sync.dma_start(xt[:], xin.ap()[128 * i:128 * (i + 1), :])
                ht = hpool.tile([128, D], BF16, tag="ht")
                ln_tile(xt[:], ht[:])
                tp = trps.tile([128, DC, 128], BF16, tag="tr")
                for dc in range(DC):
                    nc.tensor.transpose(tp[:, dc, :],
                                        ht[:, 128 * dc:128 * (dc + 1)],
                                        ident[:])
                nc.vector.tensor_copy(
                    hT[:, :, 128 * i:128 * (i + 1)], tp[:])
                if i >= 2:
                    v_chunk(i - 2, wv0_t, 0, on_act=True)
                if i in KNT_AT:
                    k_nt(KNT_AT[i], wk0_t, kp_t[0][0], kp_t[0][1])
            v_chunk(TC - 2, wv0_t, 0, on_act=True)
            v_chunk(TC - 1, wv0_t, 0, on_act=True)
            k_nt(4, wk0_t, kp_t[0][0], kp_t[0][1])

            # release prefetches now that the x tiles are in; memset gates
            # pin each DMA behind this point in the DVE stream (the
            # scheduler hoists ungated DMA issues into the phase-1 window)
            def gated_dma(dst_gate, dst, src):
                nc.vector.memset(dst_gate, 0.0)
                nc.sync.dma_start(dst, src)

            for p in range(1, NPAIR):
                gated_dma(wk_ts[p][0:1, 0:1, 0:1],
                          wk_ts[p][:].rearrange("p a b -> p (a b)"), wk.ap()[p])
                gated_dma(wq_ts[p][0:1, 0:1, 0:1],
                          wq_ts[p][:].rearrange("p a b -> p (a b)"), wq.ap()[p])
            for s in range(2):
                for ab in range(2):
                    gated_dma(kp_t[s][ab][64:65, 0:1],
                              kp_t[s][ab][64:66, :], augk.ap())
                    gated_dma(qp_t[s][ab][64:65, 0:1],
                              qp_t[s][ab][64:66, :], augq.ap())
            gated_dma(dm[0:1, 0:1, 0:1], dm[:], dm_v)
            gated_dma(bq_sb[0:1, 0:1], bq_sb[:], bq.ap())

            # ===== Phase 2: per-pair QKV + attention (+ V-half-1) =====
            def pair(p, wk_t, wq_t):
                s = p % 2
                kpA, kpB = kp_t[s]
                qpA, qpB = qp_t[s]
                if p > 0:
                    for ti in range(len(NT)):
                        k_nt(ti, wk_t, kpA, kpB)
                q_proj(p, wq_t, qpA, qpB)
                attention(p, kpA, qpA, 0)
                attention(p, kpB, qpB, 1)

            # weight streams on the SP queue (WAR-stalled prefetches must
            # not block the gpsimd queue, which carries the attention
            # partition_broadcasts)
            pair(0, wk_ts[0], wq_ts[0])
            gated_dma(wv1_t[0:1, 0:1, 0:1], wv1_t[:], wv_v[:, :, 512:1024])
            pair(1, wk_ts[1], wq_ts[1])
            gated_dma(wp_sb[0:1, 0:1, 0:1], wp_sb[:], wp_v)
            pair(2, wk_ts[2], wq_ts[2])
            gated_dma(b1_sb[0:1, 0:1], b1_sb[:], b1_v)
            gated_dma(b2_row[0:1, 0:1], b2_row[:], b2_v)
            nc.gpsimd.partition_broadcast(b2_bc[:], b2_row[0:1, :])
            pair(3, wk_ts[3], wq_ts[3])
            for tk in range(TC):
                v_chunk(tk, wv1_t, 1)
            for p in range(4, NPAIR):
                pair(p, wk_ts[p], wq_ts[p])

        attnbig.__exit__(None, None, None)

        # ===== Phase 3+4 fused: per token tile, Wp proj + residual + LN2
        # -> h2T (PSUM: 2+2 = 4 banks) =====
        with tc.tile_pool(name="ph3big", bufs=1) as ph3big, \
             tc.tile_pool(name="ph5big", bufs=1) as ph5big:
            xmid = ph3big.tile([128, 4, D], F32)
            h2T = ph5big.tile([128, DC, QL], BF16)
            with tc.tile_pool(name="xqp", bufs=3) as xqp, \
                 tc.tile_pool(name="hpool2", bufs=2) as hpool2, \
                 tc.tile_pool(name="f3ps", bufs=2, space="PSUM") as f3ps, \
                 tc.tile_pool(name="trps2", bufs=2, space="PSUM") as trps2:
                for t in range(4):
                    for dh in range(2):
                        ps = f3ps.tile([128, 512], F32, tag="f3")
                        for dc in range(DC):
                            nc.tensor.matmul(
                                ps[:], oT[:, dc, 128 * t:128 * (t + 1)],
                                wp_sb[:, dc, 512 * dh:512 * (dh + 1)],
                                start=(dc == 0), stop=(dc == DC - 1))
                        xqt = xqp.tile([128, 512], F32, tag="xqt")
                        nc.sync.dma_start(
                            xqt[:],
                            xqbp.ap()[128 * t:128 * (t + 1),
                                      512 * dh:512 * (dh + 1)])
                        nc.vector.tensor_add(
                            xmid[:, t, 512 * dh:512 * (dh + 1)], ps[:], xqt[:])
                    ht2 = hpool2.tile([128, D], BF16, tag="h2t")
                    ln_tile(xmid[:, t, :], ht2[:])
                    tp = trps2.tile([128, DC, 128], BF16, tag="tr2")
                    for dc in range(DC):
                        nc.tensor.transpose(tp[:, dc, :],
                                            ht2[:, 128 * dc:128 * (dc + 1)],
                                            ident[:])
                    nc.vector.tensor_copy(
                        h2T[:, :, 128 * t:128 * (t + 1)], tp[:])

            # ===== Phase 5: MLP + residual + output =====
            mT = ph5big.tile([128, FC, QL], BF16)
            xmb = ph5big.tile([128, 4, D], F32)
            with tc.tile_pool(name="w1p", bufs=3) as w1p, \
                 tc.tile_pool(name="w2p", bufs=4) as w2p, \
                 tc.tile_pool(name="opool", bufs=3) as opool, \
                 tc.tile_pool(name="finps2", bufs=1, space="PSUM") as finps2, \
                 tc.tile_pool(name="mps", bufs=2, space="PSUM") as mps:

                def w1_gelu(fc):
                    w1t = w1p.tile([128, DC, 128], BF16, tag="w1t")
                    nc.sync.dma_start(w1t[:].rearrange("p a b -> p (a b)"),
                                      w1.ap()[fc])
                    mp = mps.tile([128, QL], F32, tag="mp")
                    if fc < 6:
                        # token-sliced: each 128-col slice only needs that
                        # token tile's h2T, so W1 overlaps the LN2 chains of
                        # later tiles (slice starts re-arm the bank, but each
                        # prior slice is complete and only read afterward)
                        for t in range(4):
                            for dc in range(DC):
                                nc.tensor.matmul(
                                    mp[:, 128 * t:128 * (t + 1)],
                                    w1t[:, dc, :],
                                    h2T[:, dc, 128 * t:128 * (t + 1)],
                                    start=(dc == 0), stop=(dc == DC - 1),
                                    skip_group_check=True)
                    else:
                        for dc in range(DC):
                            nc.tensor.matmul(mp[:], w1t[:, dc, :],
                                             h2T[:, dc, :],
                                             start=(dc == 0),
                                             stop=(dc == DC - 1))
                    nc.scalar.activation(mT[:, fc, :], mp[:], AF.Gelu,
                                         bias=b1_sb[:, fc:fc + 1])

                def w2_acc(fc, dh, pss):
                    w2t = w2p.tile([128, 512], BF16, tag="w2t")
                    nc.sync.dma_start(
                        w2t[:], w2.ap()[128 * fc:128 * (fc + 1),
                                        512 * dh:512 * (dh + 1)])
                    for t in range(4):
                        nc.tensor.matmul(pss[t][:],
                                         mT[:, fc, 128 * t:128 * (t + 1)],
                                         w2t[:], start=(fc == 0),
                                         stop=(fc == FC - 1))

                for dh in range(2):
                    pss = [finps2.tile([128, 512], F32, tag=f"fo{t}",
                                       name=f"fo{t}_{dh}")
                           for t in range(4)]
                    if dh == 0:
                        # W2(fc) trails W1(fc+1) so gelu latency hides
                        w1_gelu(0)
                        for fc in range(1, FC):
                            w1_gelu(fc)
                            w2_acc(fc - 1, 0, pss)
                            if fc <= 4:  # xmb precompute in the shadow
                                nc.vector.tensor_add(
                                    xmb[:, fc - 1, :], xmid[:, fc - 1, :],
                                    b2_bc[:])
                        w2_acc(FC - 1, 0, pss)
                    else:
                        for fc in range(FC):
                            w2_acc(fc, 1, pss)
                    for t in range(4):
                        ot = opool.tile([128, 512], F32, tag="ot")
                        nc.vector.tensor_add(
                            ot[:], pss[t][:],
                            xmb[:, t, 512 * dh:512 * (dh + 1)])
                        nc.sync.dma_start(
                            out.ap()[128 * t:128 * (t + 1),
                                     512 * dh:512 * (dh + 1)], ot[:])

    nc.compile()
    return nc


_NC_CACHE = {}


def _get_nc():
    if "nc" not in _NC_CACHE:
        _NC_CACHE["nc"] = build_nc()
    return _NC_CACHE["nc"]


def _bf16(a):
    import ml_dtypes
    return np.ascontiguousarray(a.astype(ml_dtypes.bfloat16))


def _host_pack(inputs):
    x = np.ascontiguousarray(np.asarray(inputs["x"], dtype=np.float32))
    Wq = np.asarray(inputs["Wq"], np.float32)   # [H, D, DH]
    Wk = np.asarray(inputs["Wk"], np.float32)
    Wv = np.asarray(inputs["Wv"], np.float32)
    Wp = np.asarray(inputs["Wp"], np.float32)
    bp = np.asarray(inputs["bp"], np.float32)
    W1 = np.asarray(inputs["W1"], np.float32)
    b1 = np.asarray(inputs["b1"], np.float32)
    W2 = np.asarray(inputs["W2"], np.float32)
    b2 = np.asarray(inputs["b2"], np.float32)
    g1 = np.asarray(inputs["g1"], np.float32)
    be1 = np.asarray(inputs["be1"], np.float32)
    g2 = np.asarray(inputs["g2"], np.float32)
    be2 = np.asarray(inputs["be2"], np.float32)

    scale = np.float32(np.float64(D) ** -0.5)  # 1/32, exact power of two

    # feature-major weight matrices [D, H*DH], g1 folded in
    wq_m = (Wq * g1[None, :, None]).transpose(1, 0, 2).reshape(D, D) * scale
    wk_m = (Wk * g1[None, :, None]).transpose(1, 0, 2).reshape(D, D)
    wv_m = (Wv * g1[None, :, None]).transpose(1, 0, 2).reshape(D, D)
    # reorder wv columns: per half, even (A) heads then odd (B) heads
    horder = [0, 2, 4, 6, 1, 3, 5, 7, 8, 10, 12, 14, 9, 11, 13, 15]
    wv_r = wv_m.reshape(D, H, DH)[:, horder, :].reshape(D, D)

    # q bias (k bias is softmax-invariant; v bias folds into bp)
    bq_h = (be1 @ Wq.transpose(1, 0, 2).reshape(D, D)).reshape(H, DH) * scale
    bq_arr = np.zeros((128, NPAIR), np.float32)
    for p in range(NPAIR):
        bq_arr[0:64, p] = bq_h[2 * p]
        bq_arr[64:128, p] = bq_h[2 * p + 1]

    bv_concat = (be1 @ Wv.transpose(1, 0, 2).reshape(D, D))  # [D], orig order
    bp_eff = (bp + bv_concat @ Wp).astype(np.float32)

    w1_p = W1 * g2[:, None]
    b1_p = (b1 + be2 @ W1).astype(np.float32)

    augq = np.zeros((2, QL), np.float32)
    augq[0, 0:256] = 1.0
    augq[1, 256:512] = 1.0

    # diag masks: additive, 0 keep / -BIG drop; causal within own blocks
    dmaskv = np.empty((4, 128, 256), np.float32)
    ii = np.arange(128)
    jj = np.arange(256)
    for ci in range(4):
        loc = 128 * (ci % 2) + ii[:, None]
        keep = loc <= jj[None, :]
        dmaskv[ci] = np.where(keep, 0.0, -BIG)

    DCn, FCn, NP = D // 128, FF // 128, NPAIR
    wq_r = wq_m.reshape(DCn, 128, NP, 128).transpose(2, 1, 0, 3) \
        .reshape(NP, 128, DCn * 128)
    wk_r2 = wk_m.reshape(DCn, 128, NP, 128).transpose(2, 1, 0, 3) \
        .reshape(NP, 128, DCn * 128)
    w1_r = w1_p.reshape(DCn, 128, FCn, 128).transpose(2, 1, 0, 3) \
        .reshape(FCn, 128, DCn * 128)
    identm = np.eye(128, dtype=np.float32)
    shared = dict(wq=_bf16(wq_r), wk=_bf16(wk_r2), wv=_bf16(wv_r),
                  bq=bq_arr, wp=_bf16(Wp), w1=_bf16(w1_r), b1v=b1_p,
                  w2=_bf16(W2), b2v=b2, augq=_bf16(augq), identm=_bf16(identm),
                  dmask=np.ascontiguousarray(dmaskv))

    in_maps = []
    for c in range(NCORES):
        b, j = c // 4, c % 4
        xown = np.concatenate([x[b, 256 * j:256 * (j + 1)],
                               x[b, 256 * (7 - j):256 * (8 - j)]], axis=0)
        xin_c = np.concatenate([x[b, :KV], xown], axis=0)
        augk = np.zeros((2, T), np.float32)
        augk[0, 256 * j:KV] = -BIG      # A rect validity: t < 256j
        augk[0, KV + 256:T] = -BIG      # B-own slots never feed A cols
        augk[1, 256 * (7 - j):KV] = -BIG  # B rect validity: t < 256(7-j)
        augk[1, KV:KV + 256] = -BIG     # A-own slots already counted via rect
        in_maps.append(dict(shared, xin=_bf16(xin_c),
                            xqbp=np.ascontiguousarray(xown + bp_eff),
                            augk=_bf16(augk)))
    return x, in_maps


def _unshard(results):
    out = np.empty((B, P, D), np.float32)
    for c in range(NCORES):
        b, j = c // 4, c % 4
        o = results[c]["out"]
        out[b, 256 * j:256 * (j + 1)] = o[0:256]
        out[b, 256 * (7 - j):256 * (8 - j)] = o[256:512]
    return out


def kernel(**inputs):
    x, in_maps = _host_pack(inputs)
    nc = _get_nc()
    res = run_bass_kernel_spmd(nc, in_maps, core_ids=list(range(NCORES)))
    return _unshard(res.results)
